# revision 3
# baseline (speedup 1.0000x reference)
import sys

sys.path.insert(0, "/opt/trn_rl_repo")
import numpy as np
import ml_dtypes

NCORES = 8
BN_EPS = 1e-5
NEG = 0.2

_TIME_NS = [0]
_LAST = {}


def _split_waits(nc, mybir):
    # This walrus build allows only one sync-wait command per instruction;
    # hoist extras onto dedicated nop carriers placed just before.
    blocks = nc.main_func.blocks
    for bb in blocks:
        orig = list(bb.instructions)
        if not any(
            ins.sync_info is not None and len(ins.sync_info.on_wait) > 1
            for ins in orig
        ):
            continue
        out = []
        for ins in orig:
            si = ins.sync_info
            if si is not None and len(si.on_wait) > 1:
                waits = list(si.on_wait)
                for w in waits[:-1]:
                    nop = nc.engines[ins.engine].nop(nofuse=True, hint="waitsplit")
                    ni = nop.ins
                    popped = False
                    for b2 in blocks:
                        if b2.instructions and b2.instructions[-1] is ni:
                            b2.instructions.pop()
                            popped = True
                            break
                    assert popped, "waitsplit nop not found at block tail"
                    nsi = ni.sync_info
                    if nsi is None:
                        ni.sync_info = mybir.SyncInfo(on_wait=[w], on_update=[])
                    else:
                        nsi.on_wait = [w]
                    out.append(ni)
                si.on_wait = [waits[-1]]
            out.append(ins)
        bb.instructions[:] = out


def _bin_pack(deg, nbins, cap_nodes=128):
    """Assign each node to a bin, balancing total degree; <=cap_nodes per bin."""
    import heapq

    n = len(deg)
    order = np.argsort(-deg, kind="stable")
    heap = [(0, 0, b) for b in range(nbins)]
    heapq.heapify(heap)
    assign = np.empty(n, np.int64)
    for node in order:
        d = int(deg[node])
        spill = []
        while True:
            load, cnt, b = heapq.heappop(heap)
            if cnt < cap_nodes:
                break
            spill.append((load, cnt, b))
        assign[node] = b
        heapq.heappush(heap, (load + d, cnt + 1, b))
        # bins that hit cap_nodes stay out of the heap for good
        for s in spill:
            if s[1] < cap_nodes:
                heapq.heappush(heap, s)
    return assign


def _build_program(T, NB, NPAD, bass, mybir, tile):
    """One SPMD program for all 8 cores.

    T: edge tiles per bin (self-loops handled densely); NB: bins per core;
    NPAD: padded node count.
    """
    f32 = mybir.dt.float32
    bf16 = mybir.dt.bfloat16
    i32 = mybir.dt.int32
    NCH = NPAD // 128
    MY = NB * 128
    LR = mybir.ActivationFunctionType
    ALU = mybir.AluOpType

    nc = bass.Bass(num_devices=NCORES)

    # ---- inputs ----
    t_xT = nc.dram_tensor("xT", [NCH, 128, 128], bf16, kind="ExternalInput")
    t_xTm = nc.dram_tensor("xTm", [NB, 128, 128], bf16, kind="ExternalInput")
    t_w1l = nc.dram_tensor("w1l", [128, 256], bf16, kind="ExternalInput")
    t_w1r = nc.dram_tensor("w1r", [128, 256], bf16, kind="ExternalInput")
    t_w2 = nc.dram_tensor("w2", [2, 128, 128], bf16, kind="ExternalInput")
    t_att1 = nc.dram_tensor("att1", [128, 256], bf16, kind="ExternalInput")
    t_att2 = nc.dram_tensor("att2", [128, 64], bf16, kind="ExternalInput")
    t_abn = nc.dram_tensor("abn", [128, 256], bf16, kind="ExternalInput")
    t_bbn = nc.dram_tensor("bbn", [128, 256], bf16, kind="ExternalInput")
    t_b2 = nc.dram_tensor("b2", [128, 40], f32, kind="ExternalInput")
    t_il = nc.dram_tensor("il", [128, NB * T], i32, kind="ExternalInput")
    t_dlc = nc.dram_tensor("dlc", [128, NB * T], f32, kind="ExternalInput")
    t_iotc = nc.dram_tensor("iotc", [128, 1], f32, kind="ExternalInput")
    t_iotr = nc.dram_tensor("iotr", [128, 128], f32, kind="ExternalInput")
    t_ident = nc.dram_tensor("ident", [128, 128], f32, kind="ExternalInput")
    t_identb = nc.dram_tensor("identb", [128, 128], bf16, kind="ExternalInput")

    # ---- internal ----
    t_xl = nc.dram_tensor("xl_perm", [NPAD, 256], bf16, kind="Internal")
    t_xrm = nc.dram_tensor("xr_mine", [MY, 256], bf16, kind="Internal")
    t_hlm = nc.dram_tensor("hl_mine", [MY, 64], bf16, kind="Internal")
    t_hrm = nc.dram_tensor("hr_mine", [MY, 64], bf16, kind="Internal")
    t_hl = nc.dram_tensor(
        "hl_perm", [NPAD, 64], bf16, kind="Internal", addr_space="Shared"
    )

    # ---- output ----
    t_out = nc.dram_tensor("out_mine", [NB, 128, 40], f32, kind="ExternalOutput")

    with tile.TileContext(nc) as tc:
        with (
            tc.tile_pool(name="const", bufs=1) as cpool,
            tc.tile_pool(name="work", bufs=6) as pool,
            tc.tile_pool(name="dma2", bufs=2) as dpool,
            tc.tile_pool(name="psA", bufs=3, space="PSUM") as psA,
            tc.tile_pool(name="psB", bufs=2, space="PSUM") as psB,
            tc.tile_pool(name="psC", bufs=2, space="PSUM") as psC,
        ):
            # constants
            w1l_sb = cpool.tile([128, 256], bf16)
            nc.sync.dma_start(w1l_sb[:], t_w1l[:])
            w1r_sb = cpool.tile([128, 256], bf16)
            nc.sync.dma_start(w1r_sb[:], t_w1r[:])
            w2_sb = cpool.tile([128, 256], bf16)
            nc.sync.dma_start(w2_sb[:, 0:128], t_w2[0])
            nc.sync.dma_start(w2_sb[:, 128:256], t_w2[1])
            att1_sb = cpool.tile([128, 256], bf16)
            nc.sync.dma_start(att1_sb[:], t_att1[:])
            att2_sb = cpool.tile([128, 64], bf16)
            nc.sync.dma_start(att2_sb[:], t_att2[:])
            abn_sb = cpool.tile([128, 256], bf16)
            nc.sync.dma_start(abn_sb[:], t_abn[:])
            bbn_sb = cpool.tile([128, 256], bf16)
            nc.sync.dma_start(bbn_sb[:], t_bbn[:])
            b2_sb = cpool.tile([128, 40], f32)
            nc.sync.dma_start(b2_sb[:], t_b2[:])
            il_sb = cpool.tile([128, NB * T], i32)
            nc.sync.dma_start(il_sb[:], t_il[:])
            dlc_sb = cpool.tile([128, NB * T], f32)
            nc.sync.dma_start(dlc_sb[:], t_dlc[:])
            iotc_sb = cpool.tile([128, 1], f32)
            nc.sync.dma_start(iotc_sb[:], t_iotc[:])
            iotr_sb = cpool.tile([128, 128], f32)
            nc.sync.dma_start(iotr_sb[:], t_iotr[:])
            ident_sb = cpool.tile([128, 128], f32)
            nc.sync.dma_start(ident_sb[:], t_ident[:])
            identb_sb = cpool.tile([128, 128], bf16)
            nc.sync.dma_start(identb_sb[:], t_identb[:])

            # caches
            xlc = cpool.tile([128, NB * 256], bf16)  # own-bin xl rows
            lgs1 = cpool.tile([128, NB * 8], f32)  # self logits L1
            lgs2 = cpool.tile([128, NB], f32)  # self logits L2
            tsc = cpool.tile([128, NB * 40], f32)  # pre-logsoftmax ts
            ssc = cpool.tile([128, NB], f32)  # softmax sums

            # ---- D1a: xl table for ALL chunks (replicated) ----
            for k in range(NCH):
                xt = dpool.tile([128, 128], bf16, tag="xt")
                nc.sync.dma_start(xt[:], t_xT[k])
                ps = psA.tile([128, 256], f32, tag="mm")
                nc.tensor.matmul(
                    ps[:], lhsT=xt[:], rhs=w1l_sb[:], start=True, stop=True
                )
                sb = pool.tile([128, 256], bf16, tag="d1sb")
                nc.vector.tensor_copy(sb[:], ps[:])
                nc.sync.dma_start(t_xl[k * 128 : (k + 1) * 128, :], sb[:])

            # ---- D1b: own bins: xl cache + xr store + self logits L1 ----
            for b in range(NB):
                xt = dpool.tile([128, 128], bf16, tag="xt")
                nc.sync.dma_start(xt[:], t_xTm[b])
                psl = psA.tile([128, 256], f32, tag="mm")
                nc.tensor.matmul(
                    psl[:], lhsT=xt[:], rhs=w1l_sb[:], start=True, stop=True
                )
                psr = psA.tile([128, 256], f32, tag="mm")
                nc.tensor.matmul(
                    psr[:], lhsT=xt[:], rhs=w1r_sb[:], start=True, stop=True
                )
                nc.vector.tensor_copy(xlc[:, b * 256 : (b + 1) * 256], psl[:])
                xrs = pool.tile([128, 256], bf16, tag="xrs")
                nc.vector.tensor_copy(xrs[:], psr[:])
                nc.sync.dma_start(t_xrm[b * 128 : (b + 1) * 128, :], xrs[:])
                ts0 = pool.tile([128, 256], f32, tag="ts0")
                nc.vector.tensor_add(
                    ts0[:], xlc[:, b * 256 : (b + 1) * 256], psr[:]
                )
                lr0 = pool.tile([128, 256], bf16, tag="lr0")
                nc.vector.scalar_tensor_tensor(
                    out=lr0[:], in0=ts0[:], scalar=NEG, in1=ts0[:],
                    op0=ALU.mult, op1=ALU.max,
                )
                lm0 = pool.tile([128, 256], bf16, tag="lm0")
                nc.vector.tensor_tensor(
                    out=lm0[:], in0=lr0[:], in1=att1_sb[:], op=ALU.mult
                )
                nc.vector.tensor_reduce(
                    out=lgs1[:, b * 8 : (b + 1) * 8].rearrange(
                        "p (h o) -> p h o", o=1
                    ),
                    in_=lm0[:].rearrange("p (h c) -> p h c", h=8),
                    axis=mybir.AxisListType.X,
                    op=ALU.add,
                )

            # ---- E1: layer-1 edge pass ----
            for b in range(NB):
                xrb = dpool.tile([128, 256], bf16, tag="xrb")
                nc.sync.dma_start(xrb[:], t_xrm[b * 128 : (b + 1) * 128, :])
                acc = psB.tile([128, 264], f32, tag="acc1")
                for t in range(T):
                    col = b * T + t
                    gl = pool.tile([128, 256], bf16, tag="gl")
                    nc.gpsimd.indirect_dma_start(
                        out=gl[:],
                        out_offset=None,
                        in_=t_xl[:],
                        in_offset=bass.IndirectOffsetOnAxis(
                            ap=il_sb[:, col : col + 1], axis=0
                        ),
                    )
                    dtp = psC.tile([128, 128], f32, tag="dtp")
                    nc.tensor.transpose(
                        out=dtp[:],
                        in_=dlc_sb[:, col : col + 1].to_broadcast([128, 128]),
                        identity=ident_sb[:],
                    )
                    mt = pool.tile([128, 128], bf16, tag="mt")
                    nc.vector.tensor_tensor(
                        out=mt[:],
                        in0=iotc_sb[:].to_broadcast([128, 128]),
                        in1=dtp[:],
                        op=ALU.is_equal,
                    )
                    xre = psA.tile([128, 256], f32, tag="mm")
                    nc.tensor.matmul(
                        xre[:], lhsT=mt[:], rhs=xrb[:], start=True, stop=True
                    )
                    tt = pool.tile([128, 256], bf16, tag="tt")
                    nc.vector.tensor_add(tt[:], gl[:], xre[:])
                    lr = pool.tile([128, 256], bf16, tag="lr")
                    nc.vector.scalar_tensor_tensor(
                        out=lr[:], in0=tt[:], scalar=NEG, in1=tt[:],
                        op0=ALU.mult, op1=ALU.max,
                    )
                    lm = pool.tile([128, 256], bf16, tag="lm")
                    nc.vector.tensor_tensor(
                        out=lm[:], in0=lr[:], in1=att1_sb[:], op=ALU.mult
                    )
                    lg = pool.tile([128, 8], f32, tag="lg")
                    nc.vector.tensor_reduce(
                        out=lg[:].rearrange("p (h o) -> p h o", o=1),
                        in_=lm[:].rearrange("p (h c) -> p h c", h=8),
                        axis=mybir.AxisListType.X,
                        op=ALU.add,
                    )
                    pp = pool.tile([128, 8], bf16, tag="pp")
                    nc.scalar.activation(pp[:], lg[:], LR.Exp)
                    vp = pool.tile([128, 264], bf16, tag="vp")
                    nc.vector.tensor_tensor(
                        out=vp[:, 0:256].rearrange("p (h c) -> p h c", h=8),
                        in0=gl[:].rearrange("p (h c) -> p h c", h=8),
                        in1=pp[:]
                        .rearrange("p (h o) -> p h o", o=1)
                        .to_broadcast([128, 8, 32]),
                        op=ALU.mult,
                    )
                    nc.vector.tensor_copy(vp[:, 256:264], pp[:])
                    m1 = pool.tile([128, 128], bf16, tag="m1")
                    nc.vector.tensor_tensor(
                        out=m1[:],
                        in0=dlc_sb[:, col : col + 1].to_broadcast([128, 128]),
                        in1=iotr_sb[:],
                        op=ALU.is_equal,
                    )
                    nc.tensor.matmul(
                        acc[:], lhsT=m1[:], rhs=vp[:], start=(t == 0), stop=False
                    )
                # self-loop contribution (dense)
                pss = pool.tile([128, 8], bf16, tag="pp")
                nc.scalar.activation(pss[:], lgs1[:, b * 8 : (b + 1) * 8], LR.Exp)
                vps = pool.tile([128, 264], bf16, tag="vp")
                nc.vector.tensor_tensor(
                    out=vps[:, 0:256].rearrange("p (h c) -> p h c", h=8),
                    in0=xlc[:, b * 256 : (b + 1) * 256].rearrange(
                        "p (h c) -> p h c", h=8
                    ),
                    in1=pss[:]
                    .rearrange("p (h o) -> p h o", o=1)
                    .to_broadcast([128, 8, 32]),
                    op=ALU.mult,
                )
                nc.vector.tensor_copy(vps[:, 256:264], pss[:])
                nc.tensor.matmul(
                    acc[:], lhsT=identb_sb[:], rhs=vps[:], start=False, stop=True
                )
                # ---- evict: softmax divide, BN, ELU ----
                rden = pool.tile([128, 8], f32, tag="rden")
                nc.vector.reciprocal(rden[:], acc[:, 256:264])
                q = pool.tile([128, 256], bf16, tag="q")
                nc.vector.tensor_tensor(
                    out=q[:].rearrange("p (h c) -> p h c", h=8),
                    in0=acc[:, 0:256].rearrange("p (h c) -> p h c", h=8),
                    in1=rden[:]
                    .rearrange("p (h o) -> p h o", o=1)
                    .to_broadcast([128, 8, 32]),
                    op=ALU.mult,
                )
                h1 = pool.tile([128, 256], bf16, tag="h1")
                nc.vector.tensor_tensor(out=h1[:], in0=q[:], in1=abn_sb[:], op=ALU.mult)
                h2 = pool.tile([128, 256], bf16, tag="h2")
                nc.vector.tensor_tensor(out=h2[:], in0=h1[:], in1=bbn_sb[:], op=ALU.add)
                hneg = pool.tile([128, 256], bf16, tag="hneg")
                nc.vector.tensor_scalar(
                    out=hneg[:], in0=h2[:], scalar1=0.0, scalar2=None, op0=ALU.min
                )
                hexp = pool.tile([128, 256], bf16, tag="hexp")
                nc.scalar.activation(hexp[:], hneg[:], LR.Exp)
                hpos = pool.tile([128, 256], bf16, tag="hpos")
                nc.vector.tensor_scalar(
                    out=hpos[:], in0=h2[:], scalar1=0.0, scalar2=None, op0=ALU.max
                )
                h3 = pool.tile([128, 256], bf16, tag="h3")
                nc.vector.scalar_tensor_tensor(
                    out=h3[:], in0=hexp[:], scalar=-1.0, in1=hpos[:],
                    op0=ALU.add, op1=ALU.add,
                )
                # ---- fused D2: hl/hr for this bin ----
                ht0p = psC.tile([128, 128], bf16, tag="dtp")
                nc.tensor.transpose(
                    out=ht0p[:], in_=h3[:, 0:128], identity=identb_sb[:]
                )
                ht0 = pool.tile([128, 128], bf16, tag="ht0")
                nc.vector.tensor_copy(ht0[:], ht0p[:])
                ht1p = psC.tile([128, 128], bf16, tag="dtp")
                nc.tensor.transpose(
                    out=ht1p[:], in_=h3[:, 128:256], identity=identb_sb[:]
                )
                ht1 = pool.tile([128, 128], bf16, tag="ht1")
                nc.vector.tensor_copy(ht1[:], ht1p[:])
                hlr = psA.tile([128, 256], f32, tag="mm")
                nc.tensor.matmul(
                    hlr[:, 0:128],
                    lhsT=ht0[:],
                    rhs=w2_sb[:, 0:128],
                    start=True,
                    stop=False,
                )
                nc.tensor.matmul(
                    hlr[:, 0:128],
                    lhsT=ht1[:],
                    rhs=w2_sb[:, 128:256],
                    start=False,
                    stop=True,
                )
                hlrs = pool.tile([128, 128], bf16, tag="hlrs")
                nc.vector.tensor_copy(hlrs[:], hlr[:, 0:128])
                nc.vector.memset(hlrs[:, 40:41], 1.0)
                nc.sync.dma_start(t_hlm[b * 128 : (b + 1) * 128, :], hlrs[:, 0:64])
                hrs = pool.tile([128, 64], bf16, tag="hrs")
                nc.vector.tensor_copy(hrs[:], hlrs[:, 64:128])
                nc.sync.dma_start(t_hrm[b * 128 : (b + 1) * 128, :], hrs[:])
                # self logits for layer 2 (hl + hr of same node)
                ts2 = pool.tile([128, 64], bf16, tag="ts2")
                nc.vector.tensor_add(ts2[:], hlrs[:, 0:64], hlrs[:, 64:128])
                lr2s = pool.tile([128, 64], bf16, tag="lr2s")
                nc.vector.scalar_tensor_tensor(
                    out=lr2s[:], in0=ts2[:], scalar=NEG, in1=ts2[:],
                    op0=ALU.mult, op1=ALU.max,
                )
                lm2s = pool.tile([128, 64], bf16, tag="lm2s")
                nc.vector.tensor_tensor(
                    out=lm2s[:], in0=lr2s[:], in1=att2_sb[:], op=ALU.mult
                )
                nc.vector.tensor_reduce(
                    out=lgs2[:, b : b + 1].rearrange("p (h o) -> p h o", o=1),
                    in_=lm2s[:].rearrange("p (h c) -> p h c", h=1),
                    axis=mybir.AxisListType.X,
                    op=ALU.add,
                )

            # ---- AllGather hl ----
            nc.gpsimd.collective_compute(
                "AllGather",
                mybir.AluOpType.bypass,
                replica_groups=[list(range(NCORES))],
                ins=[t_hlm[:]],
                outs=[t_hl[:]],
            )

            # ---- E2: layer-2 edge pass ----
            for b in range(NB):
                hrb = dpool.tile([128, 64], bf16, tag="hrb")
                nc.sync.dma_start(hrb[:], t_hrm[b * 128 : (b + 1) * 128, :])
                hlb = dpool.tile([128, 64], bf16, tag="hlb")
                nc.sync.dma_start(hlb[:], t_hlm[b * 128 : (b + 1) * 128, :])
                acc2 = psB.tile([128, 41], f32, tag="acc1")
                for t in range(T):
                    col = b * T + t
                    ghl = pool.tile([128, 64], bf16, tag="ghl")
                    nc.gpsimd.indirect_dma_start(
                        out=ghl[:],
                        out_offset=None,
                        in_=t_hl[:],
                        in_offset=bass.IndirectOffsetOnAxis(
                            ap=il_sb[:, col : col + 1], axis=0
                        ),
                    )
                    dtp = psC.tile([128, 128], f32, tag="dtp")
                    nc.tensor.transpose(
                        out=dtp[:],
                        in_=dlc_sb[:, col : col + 1].to_broadcast([128, 128]),
                        identity=ident_sb[:],
                    )
                    mt = pool.tile([128, 128], bf16, tag="mt")
                    nc.vector.tensor_tensor(
                        out=mt[:],
                        in0=iotc_sb[:].to_broadcast([128, 128]),
                        in1=dtp[:],
                        op=ALU.is_equal,
                    )
                    hre = psA.tile([128, 256], f32, tag="mm")
                    nc.tensor.matmul(
                        hre[:, 0:64], lhsT=mt[:], rhs=hrb[:], start=True, stop=True
                    )
                    t2 = pool.tile([128, 64], bf16, tag="t2")
                    nc.vector.tensor_add(t2[:], ghl[:], hre[:, 0:64])
                    lr2 = pool.tile([128, 64], bf16, tag="lr2")
                    nc.vector.scalar_tensor_tensor(
                        out=lr2[:], in0=t2[:], scalar=NEG, in1=t2[:],
                        op0=ALU.mult, op1=ALU.max,
                    )
                    lm2 = pool.tile([128, 64], bf16, tag="lm2")
                    nc.vector.tensor_tensor(
                        out=lm2[:], in0=lr2[:], in1=att2_sb[:], op=ALU.mult
                    )
                    lg2 = pool.tile([128, 1], f32, tag="lg2")
                    nc.vector.tensor_reduce(
                        out=lg2[:].rearrange("p (h o) -> p h o", o=1),
                        in_=lm2[:].rearrange("p (h c) -> p h c", h=1),
                        axis=mybir.AxisListType.X,
                        op=ALU.add,
                    )
                    p2 = pool.tile([128, 1], bf16, tag="p2")
                    nc.scalar.activation(p2[:], lg2[:], LR.Exp)
                    vp2 = pool.tile([128, 41], bf16, tag="vp2")
                    nc.vector.tensor_tensor(
                        out=vp2[:],
                        in0=ghl[:, 0:41],
                        in1=p2[:].to_broadcast([128, 41]),
                        op=ALU.mult,
                    )
                    m2 = pool.tile([128, 128], bf16, tag="m1")
                    nc.vector.tensor_tensor(
                        out=m2[:],
                        in0=dlc_sb[:, col : col + 1].to_broadcast([128, 128]),
                        in1=iotr_sb[:],
                        op=ALU.is_equal,
                    )
                    nc.tensor.matmul(
                        acc2[:], lhsT=m2[:], rhs=vp2[:], start=(t == 0), stop=False
                    )
                # self-loop contribution
                p2s = pool.tile([128, 1], bf16, tag="p2")
                nc.scalar.activation(p2s[:], lgs2[:, b : b + 1], LR.Exp)
                vps2 = pool.tile([128, 41], bf16, tag="vp2")
                nc.vector.tensor_tensor(
                    out=vps2[:],
                    in0=hlb[:, 0:41],
                    in1=p2s[:].to_broadcast([128, 41]),
                    op=ALU.mult,
                )
                nc.tensor.matmul(
                    acc2[:], lhsT=identb_sb[:], rhs=vps2[:], start=False, stop=True
                )
                # ---- evict: divide, +b2, softmax stats (ln deferred) ----
                rd2 = pool.tile([128, 1], f32, tag="rd2")
                nc.vector.reciprocal(rd2[:], acc2[:, 40:41])
                o1 = pool.tile([128, 40], f32, tag="o1")
                nc.vector.tensor_tensor(
                    out=o1[:],
                    in0=acc2[:, 0:40],
                    in1=rd2[:].to_broadcast([128, 40]),
                    op=ALU.mult,
                )
                o2 = pool.tile([128, 40], f32, tag="o2")
                nc.vector.tensor_tensor(out=o2[:], in0=o1[:], in1=b2_sb[:], op=ALU.add)
                rmx = pool.tile([128, 1], f32, tag="rmx")
                nc.vector.tensor_reduce(
                    out=rmx[:].rearrange("p (h o) -> p h o", o=1),
                    in_=o2[:].rearrange("p (h c) -> p h c", h=1),
                    axis=mybir.AxisListType.X,
                    op=ALU.max,
                )
                nc.vector.tensor_tensor(
                    out=tsc[:, b * 40 : (b + 1) * 40],
                    in0=o2[:],
                    in1=rmx[:].to_broadcast([128, 40]),
                    op=ALU.subtract,
                )
                es = pool.tile([128, 40], f32, tag="es")
                nc.scalar.activation(es[:], tsc[:, b * 40 : (b + 1) * 40], LR.Exp)
                nc.vector.tensor_reduce(
                    out=ssc[:, b : b + 1].rearrange("p (h o) -> p h o", o=1),
                    in_=es[:].rearrange("p (h c) -> p h c", h=1),
                    axis=mybir.AxisListType.X,
                    op=ALU.add,
                )

            # ---- final: one Ln over all bins, subtract, store ----
            lss = cpool.tile([128, NB], f32)
            nc.scalar.activation(lss[:], ssc[:], LR.Ln)
            for b in range(NB):
                fin = pool.tile([128, 40], f32, tag="fin")
                nc.vector.tensor_tensor(
                    out=fin[:],
                    in0=tsc[:, b * 40 : (b + 1) * 40],
                    in1=lss[:, b : b + 1].to_broadcast([128, 40]),
                    op=ALU.subtract,
                )
                nc.sync.dma_start(t_out[b], fin[:])

    _split_waits(nc, mybir)
    return nc


def kernel(
    x,
    edge_index,
    W1_l,
    W1_r,
    att1,
    b1,
    bn_gamma,
    bn_beta,
    bn_mean,
    bn_var,
    W2_l,
    W2_r,
    att2,
    b2,
):
    import time
    import concourse.bass as bass
    import concourse.mybir as mybir
    import concourse.tile as tile
    from concourse.bass_utils import run_bass_kernel_spmd

    x = np.asarray(x, np.float32)
    edge_index = np.asarray(edge_index, np.int64)
    f = lambda a: np.asarray(a, np.float32)
    W1_l, W1_r, att1, b1 = f(W1_l), f(W1_r), f(att1), f(b1)
    bn_gamma, bn_beta, bn_mean, bn_var = f(bn_gamma), f(bn_beta), f(bn_mean), f(bn_var)
    W2_l, W2_r, att2, b2 = f(W2_l), f(W2_r), f(att2), f(b2)

    n = x.shape[0]
    src = edge_index[0]
    dst = edge_index[1]

    # ---- host: bin packing + permutation (self-loops handled densely) ----
    NBINS_TOTAL = ((n + 127) // 128 + NCORES - 1) // NCORES * NCORES
    NB = NBINS_TOTAL // NCORES
    NPAD = NBINS_TOTAL * 128
    deg = np.bincount(dst, minlength=n)
    assign = _bin_pack(deg, NBINS_TOTAL)
    sorted_idx = np.argsort(assign, kind="stable")
    bin_counts = np.bincount(assign, minlength=NBINS_TOTAL)
    bin_starts = np.zeros(NBINS_TOTAL + 1, np.int64)
    np.cumsum(bin_counts, out=bin_starts[1:])
    slot_in_bin = np.empty(n, np.int64)
    slot_in_bin[sorted_idx] = np.arange(n) - bin_starts[assign[sorted_idx]]
    p_of = assign * 128 + slot_in_bin

    ebin = assign[dst]
    eorder = np.argsort(ebin, kind="stable")
    bin_e_counts = np.bincount(ebin, minlength=NBINS_TOTAL)
    T = max(1, int(-(-bin_e_counts.max() // 128)))
    CAP = T * 128
    srcslot = np.zeros((NBINS_TOTAL, CAP), np.int32)
    dstloc = np.full((NBINS_TOTAL, CAP), 999.0, np.float32)
    starts = np.zeros(NBINS_TOTAL + 1, np.int64)
    np.cumsum(bin_e_counts, out=starts[1:])
    es = src[eorder]
    ed = dst[eorder]
    for b in range(NBINS_TOTAL):
        lo, hi = starts[b], starts[b + 1]
        cnt = hi - lo
        srcslot[b, :cnt] = p_of[es[lo:hi]]
        dstloc[b, :cnt] = slot_in_bin[ed[lo:hi]].astype(np.float32)

    # ---- per-core inputs ----
    xp = np.zeros((NPAD, 128), np.float32)
    xp[p_of] = x
    xT_chunks = np.ascontiguousarray(
        xp.T.reshape(128, NPAD // 128, 128).transpose(1, 0, 2)
    ).astype(ml_dtypes.bfloat16)
    a_bn = bn_gamma / np.sqrt(bn_var + BN_EPS)
    B_bn = (b1 - bn_mean) * a_bn + bn_beta
    w2comb = np.zeros((256, 128), np.float32)
    w2comb[:, 0:40] = W2_l
    w2comb[:, 64:104] = W2_r
    att2_row = np.zeros((1, 64), np.float32)
    att2_row[0, :40] = att2.reshape(-1)
    rep = lambda a, c: np.broadcast_to(a.reshape(1, c), (128, c)).copy()

    bf = ml_dtypes.bfloat16
    in_maps = []
    for c in range(NCORES):
        b0, b1_ = c * NB, (c + 1) * NB
        il = np.ascontiguousarray(srcslot[b0:b1_].reshape(NB * T, 128).T)
        dlc = np.ascontiguousarray(
            dstloc[b0:b1_].reshape(NB * T, 128).T
        ).astype(np.float32)
        in_maps.append(
            {
                "xT": xT_chunks,
                "xTm": xT_chunks[b0:b1_],
                "w1l": W1_l.astype(bf),
                "w1r": W1_r.astype(bf),
                "w2": np.ascontiguousarray(w2comb.reshape(2, 128, 128)).astype(bf),
                "att1": rep(att1, 256).astype(bf),
                "att2": rep(att2_row, 64).astype(bf),
                "abn": rep(a_bn, 256).astype(bf),
                "bbn": rep(B_bn, 256).astype(bf),
                "b2": rep(b2, 40),
                "il": il,
                "dlc": dlc,
                "iotc": np.arange(128, dtype=np.float32).reshape(128, 1),
                "iotr": np.broadcast_to(
                    np.arange(128, dtype=np.float32).reshape(1, 128), (128, 128)
                ).copy(),
                "ident": np.eye(128, dtype=np.float32),
                "identb": np.eye(128, dtype=np.float32).astype(bf),
            }
        )

    nc = _build_program(T, NB, NPAD, bass, mybir, tile)
    import os

    trace = bool(os.environ.get("KV2_TRACE"))
    t0 = time.perf_counter()
    res = run_bass_kernel_spmd(nc, in_maps, core_ids=list(range(NCORES)), trace=trace)
    _TIME_NS[0] = int((time.perf_counter() - t0) * 1e9)
    if res.exec_time_ns:
        _TIME_NS[0] = int(res.exec_time_ns)
    _LAST["res"] = res

    outp = np.concatenate(
        [res.results[c]["out_mine"].reshape(NB * 128, 40) for c in range(NCORES)], 0
    )
    return outp[p_of]


def last_device_time_ns():
    return _TIME_NS[0]
